# revision 10
# baseline (speedup 1.0000x reference)
"""Trainium2 Bass kernel: windowed mean-color similarity.

Input  frames [8, 2048, 64, 64, 3] f32  (B, T, H, W, C), lookup_window=101.
Output [8, 2048, 101] f32:
    mc[b,t]    = mean over (H,W) of frames[b,t]            # [B,T,3]
    idx(t,j)   = max(0, t-50) + j                          # window anchor
    sim[b,t,j] = 1/(1 + ||mc[b,t]-mc[b,clip(idx)]||^2)  if idx < min(T, t+51) else 0

Sharding: data-parallel along B, one batch element per NeuronCore (8 cores).
Windows run along T which is fully local per batch element -> no halo.

Per-core kernel (T=2048 rows of HWC=12288 floats, ~100 MB):
  phase 1: stream frames in 16 tiles [128, 12288], alternating the two
           HWDGE rings (sync/SP and scalar/ACT) for the 6.3 MB loads. The
           per-channel sum over hw is split across engines so no single
           engine is the bottleneck: DVE tensor_reduce takes columns
           [0, S), ACT Copy-with-accum takes [S, HWC) (3 ops, one per
           channel). Partials combine into SUMS [128, 3] -> DRAM scratch
           mc_pad (double-buffered per rep; the 1/HW^2 scale is folded
           into phase 2).
  phase 2: per tile, a diagonal access pattern DMA on the gpsimd (SWDGE)
           ring (partition p starts at row t0+p-50, 303 contiguous floats)
           gathers each row's neighbor window from mc_pad; DVE computes
           d = sum_c (ctr-nb)^2, sim = 1/(1 + d/HW^2). Only edge tiles
           (0, 15) have invalid slots: they multiply by a mask held
           resident in SBUF (loaded once). Tile 0 uses a broadcast AP for
           rows t<50 (window anchored at 0). Output store also on SWDGE,
           keeping the HWDGE rings free for frame streaming.
"""

import numpy as np

_B, _T, _H, _W, _C = 8, 2048, 64, 64, 3
_HW = _H * _W              # 4096
_HWC = _HW * _C            # 12288
_WL = 101                  # lookup window
_HALF = _WL // 2           # 50
_P = 128                   # SBUF partitions per tile
_NT = _T // _P             # 16 tiles
_S_DVE = 876               # DVE's raw column share (mult of C)
RING = "sp"              # frame DMA ring: "sp" (all sync) | "alt" (sync/scalar)
SMALL = "sync"          # small-DMA engine: "gpsimd" | "sync"
SPLIT = True               # reduce split DVE+ACT (else all-DVE)
_P_POOL = 5484             # Pool pre-add column share (0 = off; mult of 6)
_S_ACT = 5928              # ACT column share (rest goes to DVE)


def _one_pass(nc, fp, mcp, pp, pl, p2, frames, out, mc_pad, adummy, masks, T, HW, C, WL):
    """Emit one full pass (phase 1 + phase 2) into the open TileContext."""
    import bass_rust
    import concourse.mybir as mybir

    f32 = mybir.dt.float32
    HWC = HW * C
    HALF = WL // 2
    P = _P
    NT = T // P
    WLC = WL * C
    S = _S_DVE
    X = mybir.AxisListType.X
    ADD = mybir.AluOpType.add
    MULT = mybir.AluOpType.mult
    AF = mybir.ActivationFunctionType
    small = nc.gpsimd if SMALL == "gpsimd" else nc.sync

    def diag_src(offset_elems, nrows):
        # [nrows, WLC] view of mc_pad: row r starts at offset_elems + r*C
        # (overlapping windows -> custom AP, not expressible via rearrange)
        ap = mc_pad[:].copy()
        ap.ap = bass_rust.VecI64Pair([(C, nrows), (1, WLC)])
        ap.offset = offset_elems
        return ap

    # ---- phase 1: per-tile channel sums -> mc_pad ----
    # DMA issue runs LAG tiles ahead of compute so the ACT-ring dma_starts
    # (odd tiles) are not trapped behind ACT's own reduce work in program
    # order. LAG must stay < fbufs so the issuing engine never blocks on a
    # buffer-free semaphore for long.
    LAG = 2
    mcts = []
    fts = {}

    def issue(k):
        eng = nc.sync if (RING == "sp" or k % 2 == 0) else nc.scalar
        ft = fp.tile([P, HWC], f32, tag="ft")
        eng.dma_start(out=ft[:], in_=frames[k * P:(k + 1) * P, :])
        fts[k] = ft

    def compute(k):
        ft = fts.pop(k)
        mct = mcp.tile([P, C], f32, tag="mc")
        if not SPLIT:
            v = ft[:].rearrange("p (hw c) -> p c hw", c=C)
            nc.vector.tensor_reduce(out=mct[:], in_=v, axis=X, op=ADD)
        else:
            PP = _P_POOL
            # DVE share: columns [0, S)
            dpart = pp.tile([P, C], f32, tag="dpart")
            vd = ft[:, 0:S].rearrange("p (hw c) -> p c hw", c=C)
            nc.vector.tensor_reduce(out=dpart[:], in_=vd, axis=X, op=ADD)
            if PP:
                # Pool pre-adds the halves of its range (channel-aligned);
                # DVE channel-reduces the partial
                half = PP // 2
                part = pl.tile([P, half], f32, tag="part")
                nc.gpsimd.tensor_add(
                    out=part[:], in0=ft[:, S:S + half],
                    in1=ft[:, S + half:S + PP])
                ppart = pp.tile([P, C], f32, tag="ppart")
                vp = part[:].rearrange("p (hw c) -> p c hw", c=C)
                nc.vector.tensor_reduce(out=ppart[:], in_=vp, axis=X, op=ADD)
                nc.vector.tensor_add(out=dpart[:], in0=dpart[:], in1=ppart[:])
            # ACT share: columns [S+PP, HWC), one accum per channel
            apart = pp.tile([P, C], f32, tag="apart")
            va = ft[:, S + PP:].rearrange("p (hw c) -> p c hw", c=C)
            for c in range(C):
                nc.scalar.activation(
                    out=adummy[:], in_=va[:, c, :], func=AF.Copy,
                    accum_out=apart[:, c:c + 1],
                )
            nc.vector.tensor_add(out=mct[:], in0=dpart[:], in1=apart[:])
        dst = mc_pad[k * P * C:(k + 1) * P * C].rearrange("(p c) -> p c", c=C)
        small.dma_start(out=dst, in_=mct[:])
        mcts.append(mct)

    # ---- phase 2: windowed similarity (emitted LAG2 tiles behind phase 1
    # so the diag-gather chain mc write -> diag DMA -> DVE is always stale
    # by several DMA tile-times; the greedy scheduler can then interleave
    # freely without head-of-line blocking DVE on the gather latency) ----
    def ph2(k):
        t0 = k * P
        nb = p2.tile([P, WLC], f32, tag="nb")
        if k == 0:
            # rows t<HALF: window anchored at row 0 (broadcast)
            bc = mc_pad[:].copy()
            bc.ap = bass_rust.VecI64Pair([(0, HALF), (1, WLC)])
            bc.offset = 0
            small.dma_start(out=nb[0:HALF, :], in_=bc)
            small.dma_start(out=nb[HALF:P, :], in_=diag_src(0, P - HALF))
        else:
            small.dma_start(out=nb[:], in_=diag_src((t0 - HALF) * C, P))

        mct = mcts[k]
        d = p2.tile([P, WLC], f32, tag="d")
        dsum = p2.tile([P, WL], f32, tag="dsum")
        nbv = nb[:].rearrange("p (w c) -> p w c", c=C)
        dv = d[:].rearrange("p (w c) -> p w c", c=C)
        ctr = mct[:].unsqueeze(1).broadcast_to((P, WL, C))
        nc.vector.tensor_tensor(
            out=dv, in0=ctr, in1=nbv, op=mybir.AluOpType.subtract)
        nc.vector.tensor_mul(out=d[:], in0=d[:], in1=d[:])
        nc.vector.tensor_reduce(out=dsum[:], in_=dv, axis=X, op=ADD)
        # sums -> means: diff = dsum/HW^2 ; then +1
        nc.vector.tensor_scalar(
            out=dsum[:], in0=dsum[:],
            scalar1=1.0 / (HW * HW), scalar2=1.0, op0=MULT, op1=ADD,
        )
        sim = p2.tile([P, WL], f32, tag="sim")
        nc.vector.reciprocal(out=sim[:], in_=dsum[:])
        if k == 0:
            nc.vector.tensor_mul(out=sim[:], in0=sim[:], in1=masks[0][:])
        elif k == NT - 1:
            nc.vector.tensor_mul(out=sim[:], in0=sim[:], in1=masks[1][:])
        small.dma_start(out=out[t0:t0 + P, :], in_=sim[:])

    LAG2 = LAG + 3
    for k in range(NT + LAG2):
        if k < NT:
            issue(k)
        if LAG <= k < NT + LAG:
            compute(k - LAG)
        if k >= LAG2:
            ph2(k - LAG2)


def _build_nc(T, HW, C, WL, fbufs=3, reps=1):
    """Build the single-core Bass program (parametrized for small-size sim
    tests). reps>1 repeats the computation back-to-back inside one NEFF —
    benchmarking only (amortizes the ~3 ms axon dispatch RTT)."""
    import concourse.mybir as mybir
    import concourse.tile as tile
    from concourse import bacc

    f32 = mybir.dt.float32
    HWC = HW * C
    HALF = WL // 2
    P = _P
    NT = T // P
    assert T % P == 0 and HALF < P
    PAD_T = T + ((HALF + 63) // 64) * 64   # rows beyond T are zeroed, never valid

    nc = bacc.Bacc("TRN2")
    frames = nc.dram_tensor("frames", [T, HWC], f32, kind="ExternalInput")
    maskin = nc.dram_tensor("mask", [2 * P, WL], f32, kind="ExternalInput")
    out = nc.dram_tensor("out", [T, WL], f32, kind="ExternalOutput")
    mc_pads = [
        nc.dram_tensor(f"mc_pad{i}", [PAD_T * C], f32) for i in range(2)
    ]

    with tile.TileContext(nc) as tc:
        with (
            tc.tile_pool(name="fp", bufs=fbufs) as fp,
            tc.tile_pool(name="mcp", bufs=NT) as mcp,
            tc.tile_pool(name="pp", bufs=6) as pp,
            tc.tile_pool(name="pl", bufs=2) as pl,
            tc.tile_pool(name="p2", bufs=3) as p2,
            tc.tile_pool(name="misc", bufs=1) as misc,
        ):
            # zero the pad tails of the mc_pad scratches once (1-partition
            # SBUF->DRAM DMAs fail NEFF load here, so use PAD_T-T partitions
            # x C floats)
            zt = misc.tile([PAD_T - T, C], f32, tag="zt")
            nc.vector.memset(zt[:], 0.0)
            for mp in mc_pads:
                nc.gpsimd.dma_start(
                    out=mp[T * C:].rearrange("(p c) -> p c", c=C), in_=zt[:]
                )
            # resident validity masks for the two edge tiles
            mask0 = misc.tile([P, WL], f32, tag="mask0")
            mask1 = misc.tile([P, WL], f32, tag="mask1")
            masks = [mask0, mask1]
            for i in range(2):
                nc.gpsimd.dma_start(
                    out=masks[i][:], in_=maskin[i * P:(i + 1) * P, :]
                )
            # shared dummy main-out for the ACT accum reduce
            adummy = misc.tile([P, max(HWC - _S_DVE - _P_POOL, _S_ACT) // C], f32, tag="adummy")
            for rep in range(reps):
                _one_pass(nc, fp, mcp, pp, pl, p2, frames, out, mc_pads[rep % 2],
                          adummy, masks, T, HW, C, WL)

    nc.compile()
    return nc


def _valid_mask(T, WL):
    t = np.arange(T)[:, None]
    j = np.arange(WL)[None, :]
    half = WL // 2
    start = np.maximum(0, t - half)
    end = np.minimum(T, t + half + 1)
    return ((start + j) < end).astype(np.float32)


def _edge_masks(T, WL):
    m = _valid_mask(T, WL)
    return np.concatenate([m[:_P], m[T - _P:]], axis=0)


_NC_CACHE = {}


def kernel(frames, lookup_window):
    frames = np.asarray(frames, dtype=np.float32)
    lookup_window = int(lookup_window)
    assert frames.shape == (_B, _T, _H, _W, _C), frames.shape
    assert lookup_window == _WL, lookup_window

    from concourse.bass_utils import run_bass_kernel_spmd

    if "nc" not in _NC_CACHE:
        _NC_CACHE["nc"] = _build_nc(_T, _HW, _C, _WL)
    nc = _NC_CACHE["nc"]

    mask = _edge_masks(_T, _WL)
    flat = np.ascontiguousarray(frames.reshape(_B, _T, _HWC))
    in_maps = [{"frames": flat[b], "mask": mask} for b in range(_B)]
    res = run_bass_kernel_spmd(nc, in_maps, list(range(_B)))
    return np.stack([res.results[b]["out"] for b in range(_B)], axis=0)
